# revision 39
# baseline (speedup 1.0000x reference)
"""Bass/Trainium2 kernel for NF4-dequant (QLoRA-style) SwiGLU MLP.

Computation (matches the bitsandbytes-NF4 reference):
    dq_i = nf4_quant_dequant(w_i)   (per-64-block absmax scaling)
    out  = dq3-proj( silu(x @ dq1^T) * (x @ dq2^T) )

Sharding: tensor-parallel over the ffn dim H=11008 across 8 cores.
H is split in 64-aligned shards of width [1408 x4, 1344 x4]; the 1344
shards are zero-padded to 1408 so every core runs the same program.
Each core computes a full [T, D] partial of the down-projection; the
host sums the 8 partials (the TP all-reduce).

Dequant strategy (per 128 x cw natural weight tile):
  absmax reduce on GPSIMD; normalize to an int16 grid (vn = w *
  32760/absmax) on GPSIMD/DVE; the 15-boundary NF4 bucketize runs as a
  sum of step terms  acc = sum_j (vn > IB_j) * IDELT_j  with the
  compares split across three engines:
    - DVE:    fused tensor_scalar (is_gt, mult) at 4x int16 rate
    - GPSIMD: plain is_gt masks {0,1}
    - ACT:    Sign(vn - (IB_j+0.5)) giving exact {-1,+1} (deltas are
              even so the +-IDELT/2 folds stay integral)
  all folded into an int16 acc via fused scalar_tensor_tensor on DVE.
  Rescale (acc - offset) * absmax/16384 -> bf16, then XBAR DMA
  transposes (SBUF->SBUF) produce lhsT-layout tiles which are stored
  to DRAM scratch with one batched DMA (no PE transposes, no PSUM
  evict copies).

Phase2 processes token-blocks in pairs (2 x 512) so each s1/s2 strip
load feeds two blocks and each lhsT stationary load feeds 2x512
columns.  Phase3 loads each s3 slice once per (tb, dc) and shares it
across both 4-bank PSUM half-groups.

Env knobs: KSPLIT="d,g,a" (# compares on DVE/GPSIMD/ACT),
KERNEL_GPS_NORM, KERNEL_GPS_RESCALE, KERNEL_GPS_REDUCE (0|1).
"""

import os
import sys

import numpy as np

if not os.path.isdir(os.path.join(os.path.dirname(os.path.abspath(__file__)), "concourse")):
    for _p in ("/opt/trn_rl_repo",):
        if os.path.isdir(_p) and _p not in sys.path:
            sys.path.insert(0, _p)

import ml_dtypes

import concourse.bass as bass
import concourse.mybir as mybir
import concourse.tile as tile
from concourse import bacc
from concourse.bass_utils import run_bass_kernel_spmd

F32 = mybir.dt.float32
F16 = mybir.dt.float16
BF16 = mybir.dt.bfloat16
I16 = mybir.dt.int16
OP = mybir.AluOpType
AF = mybir.ActivationFunctionType

NF4_CODE = np.array(
    [
        -1.0, -0.6961928009986877, -0.5250730514526367, -0.39491748809814453,
        -0.28444138169288635, -0.18477343022823334, -0.09105003625154495, 0.0,
        0.07958029955625534, 0.16093020141124725, 0.24611230194568634,
        0.33791524171829224, 0.44070982933044434, 0.5626170039176941,
        0.7229568362236023, 1.0,
    ],
    dtype=np.float64,
)
NF4_BOUNDS = (NF4_CODE[:-1] + NF4_CODE[1:]) * 0.5

CSCALE = 32760.0  # int16 compare-domain scale (saturation-safe)
IBOUND = [int(np.floor(b * CSCALE)) for b in NF4_BOUNDS]
# Even cumulative code table at scale 16384: CUM[j] ~ (code_j + 1) * 16384.
CUM = [2 * int(round((c + 1.0) * 8192.0)) for c in NF4_CODE]
IDELT = [CUM[j + 1] - CUM[j] for j in range(15)]  # all even
VSCALE = 16384.0

BLK = 64

D = 4096
T_FULL = 4096
H_FULL = 11008
N_CORES = 8
HP = 1408
SHARD_W = [1408, 1408, 1408, 1408, 1344, 1344, 1344, 1344]
SHARD_START = [0, 1408, 2816, 4224, 5632, 6976, 8320, 9664]

KT = D // 128  # 32
HT = HP // 128  # 11
TBP = 512           # phase2 token block (processed in pairs)
NPAIR = T_FULL // (2 * TBP)  # 4
T3 = 1024           # phase3 token block
SEG = 2048          # strip segment width (bf16)

DQ_CHUNK = 1024
W3_CHUNKS = [(0, 1024), (1024, 384)]

_split = os.environ.get("KSPLIT", "12,0,3")
N_DVE, N_GPS, N_ACT = [int(v) for v in _split.split(",")]
assert N_DVE + N_GPS + N_ACT == 15
DVE_J = list(range(0, N_DVE))
GPS_J = list(range(N_DVE, N_DVE + N_GPS))
ACT_J = list(range(N_DVE + N_GPS, 15))
S_HALF = sum(IDELT[j] // 2 for j in ACT_J)
OFFSET = 16384 - S_HALF  # dq = (acc - OFFSET) * absmax/16384

# Pool (gpsimd) integer ops require matching dtypes on all operands, so the
# f32->i16 normalize and i16xf32->bf16 rescale must stay on DVE.
GPS_NORM = os.environ.get("KERNEL_GPS_NORM", "0") == "1"
GPS_RESCALE = os.environ.get("KERNEL_GPS_RESCALE", "0") == "1"
# gpsimd tensor_reduce is partition-axis only; free-dim absmax must run on DVE
GPS_REDUCE = os.environ.get("KERNEL_GPS_REDUCE", "0") == "1"


class P:
    pass


def _build_program():
    nc = bacc.Bacc("TRN2", target_bir_lowering=False, debug=False, num_devices=N_CORES)

    xT = nc.dram_tensor("xT", [D, T_FULL], BF16, kind="ExternalInput").ap()
    w1s = nc.dram_tensor("w1s", [HP, D], F32, kind="ExternalInput").ap()
    w2s = nc.dram_tensor("w2s", [HP, D], F32, kind="ExternalInput").ap()
    w3s = nc.dram_tensor("w3s", [D, HP], F32, kind="ExternalInput").ap()
    out = nc.dram_tensor("out", [T_FULL, D], F32, kind="ExternalOutput").ap()

    from contextlib import ExitStack

    with tile.TileContext(nc) as tc, ExitStack() as ctx:
        p = P()
        dram = ctx.enter_context(tc.tile_pool(name="dram", bufs=1, space="DRAM"))
        s1 = dram.tile([HT, 128, KT, 128], BF16)
        s2 = dram.tile([HT, 128, KT, 128], BF16)
        s3 = dram.tile([HT, 128, D], BF16)
        hTd = dram.tile([HT, 128, T_FULL], BF16)

        const = ctx.enter_context(tc.tile_pool(name="const", bufs=1))
        bias_aps = []
        for j in range(15):
            b = const.tile([128, 1], F32, tag=f"bias{j}", name=f"bias{j}")
            nc.vector.memset(b[:], -(IBOUND[j] + 0.5))
            bias_aps.append(b)
        from concourse.masks import make_identity
        ident = const.tile([128, 128], F16, tag="ident", name="ident")
        make_identity(nc, ident[:])
        diag_aps = {}
        for j in range(15):
            val = IDELT[j] // 2 if j in ACT_J else IDELT[j]
            dgt = const.tile([128, 128], F16, tag=f"diag{j}", name=f"diag{j}")
            nc.vector.tensor_scalar_mul(dgt[:], ident[:], float(val))
            diag_aps[j] = dgt

        pool_spec = [
            ("pxb", 64, [128, TBP], BF16),
            ("pw", 5, [128, DQ_CHUNK], F32),
            ("pa", 6, [128, DQ_CHUNK // BLK], F32),
            ("pvn", 5, [128, DQ_CHUNK], I16),
            ("pmask", 9, [128, DQ_CHUNK], BF16),
            ("pdq", 5, [128, DQ_CHUNK], BF16),
            ("pqt", 5, [128, DQ_CHUNK], BF16),
            ("pl", 3, [128, SEG], BF16),  # bufs are per-tag (l1, l2)
            ("pht", 2, [128, TBP], BF16),
            ("psl", 3, [128, TBP], BF16),
            ("pue", 2, [128, TBP], BF16),
            ("phl", 11, [128, T3], BF16),
            ("pr3", 12, [128, 512], BF16),
            ("pob", 2, [128, 512], F32),
        ]
        for nm, bufs, shape, dt in pool_spec:
            setattr(p, nm, ctx.enter_context(tc.tile_pool(name=nm, bufs=bufs)))
        p.pps = ctx.enter_context(tc.tile_pool(name="pps", bufs=8, space="PSUM"))

        # ---------------- dequant one [128, cw] natural tile ----------------
        # Stage 1 (loads + absmax + normalize) for tile n+1 is emitted before
        # stage 2 (compares/folds/rescale/store) of tile n, so ACT/PE see vn
        # one tile ahead of the DVE chain (software pipelining).
        def dq_tile_s1(w_ap, row0, col0, cw):
            nblk = cw // BLK
            wt = p.pw.tile([128, cw], F32, tag="wt", name="wt")
            qw = max(256, cw // 4)
            for c0 in range(0, cw, qw):
                ce = min(c0 + qw, cw)
                nc.sync.dma_start(
                    wt[:, c0:ce],
                    w_ap[row0 : row0 + 128, col0 + c0 : col0 + ce],
                )
            w3v = wt[:].rearrange("p (b i) -> p b i", i=BLK)

            amax = p.pa.tile([128, nblk], F32, tag="amax", name="amax")
            nc.vector.tensor_reduce(
                amax[:], w3v, axis=mybir.AxisListType.X, op=OP.max,
                apply_absolute_value=True,
            )
            aclamp = p.pa.tile([128, nblk], F32, tag="aclamp", name="aclamp")
            nc.vector.tensor_scalar_max(aclamp[:], amax[:], 1e-30)
            recip = p.pa.tile([128, nblk], F32, tag="recip", name="recip")
            nc.vector.reciprocal(recip[:], aclamp[:])
            # av must stay f32: amax/16384 ~ 3e-7 underflows fp16 normals
            av = p.pa.tile([128, nblk], F32, tag="av", name="av")
            nc.vector.tensor_scalar_mul(av[:], amax[:], 1.0 / VSCALE)

            r_b = recip[:].unsqueeze(2).broadcast_to([128, nblk, BLK])

            vn = p.pvn.tile([128, cw], I16, tag="vn", name="vn")
            vn3 = vn[:].rearrange("p (b i) -> p b i", i=BLK)
            # vn = (w * CSCALE) * (1/absmax), fused in one STT pass
            nc.vector.scalar_tensor_tensor(
                vn3, w3v, CSCALE, r_b, OP.mult, OP.mult
            )
            return (vn, av, cw)

        def dq_tile_s2(s1ctx, store_fn):
            vn, av, cw = s1ctx
            nblk = cw // BLK
            av_b = av[:].unsqueeze(2).broadcast_to([128, nblk, BLK])

            halves = [(h0, min(512, cw - h0)) for h0 in range(0, cw, 512)]
            fold_ps = [
                p.pps.tile([128, hw], F32, tag="ps", name="fold")
                for h0, hw in halves
            ]
            nterm = 15

            # DVE terms: one-op is_gt -> {0,1} bf16 masks, folded on PE with
            # diag(IDELT_j); ACT terms: Sign -> {-1,+1}, diag(IDELT_j/2).
            # Every fold is a PE matmul accumulating into PSUM; the DVE int16
            # accumulator (and its drain-heavy add chain) is gone.
            ti = 0
            for j in DVE_J:
                m = p.pmask.tile([128, cw], BF16, tag="mask", name=f"d{j}")
                nc.vector.tensor_scalar(
                    m[:], vn[:], float(IBOUND[j]), None, OP.is_gt
                )
                for (h0, hw), ps in zip(halves, fold_ps):
                    nc.tensor.matmul(
                        ps[:], diag_aps[j][:], m[:, h0 : h0 + hw],
                        start=(ti == 0), stop=(ti == nterm - 1),
                    )
                ti += 1
            for j in ACT_J:
                sg = p.pmask.tile([128, cw], BF16, tag="mask", name=f"a{j}")
                nc.scalar.activation(sg[:], vn[:], AF.Sign, bias=bias_aps[j][:])
                for (h0, hw), ps in zip(halves, fold_ps):
                    nc.tensor.matmul(
                        ps[:], diag_aps[j][:], sg[:, h0 : h0 + hw],
                        start=(ti == 0), stop=(ti == nterm - 1),
                    )
                ti += 1

            dq = p.pdq.tile([128, cw], BF16, tag="dq", name="dq")
            for (h0, hw), ps in zip(halves, fold_ps):
                nb0, nbw = h0 // BLK, hw // BLK
                dqv = dq[:, h0 : h0 + hw].rearrange("p (b i) -> p b i", i=BLK)
                psv = ps[:].rearrange("p (b i) -> p b i", i=BLK)
                nc.vector.scalar_tensor_tensor(
                    dqv, psv, float(-OFFSET), av_b[:, nb0 : nb0 + nbw, :],
                    OP.add, OP.mult,
                )
            qt = p.pqt.tile([128, cw], BF16, tag="qt", name="qt")
            for jb in range(cw // 128):
                sl = slice(jb * 128, (jb + 1) * 128)
                nc.sync.dma_start_transpose(qt[:, sl], dq[:, sl])
            store_fn(qt)

        dq_pending = []

        def dq_push(w_ap, row0, col0, cw, store_fn):
            s1ctx = dq_tile_s1(w_ap, row0, col0, cw)
            dq_pending.append((s1ctx, store_fn))
            if len(dq_pending) > 3:
                dq_tile_s2(*dq_pending.pop(0))

        def dq_drain():
            while dq_pending:
                dq_tile_s2(*dq_pending.pop(0))

        def dq_w12(which, s, h):
            w_ap = w1s if which == 1 else w2s
            for ch in range(0, D, DQ_CHUNK):
                def store(qt, ch=ch, h=h, s=s):
                    kt0 = ch // 128
                    nkt = DQ_CHUNK // 128
                    dst = s[h, :, kt0 : kt0 + nkt, :].rearrange("p k i -> p (k i)")
                    nc.gpsimd.dma_start(dst, qt[:])
                dq_push(w_ap, h * 128, ch, DQ_CHUNK, store)

        w3_work = [(i, ch, cw) for i in range(KT) for (ch, cw) in W3_CHUNKS]
        w3_iter = iter(w3_work)

        def emit_w3(n):
            for _ in range(n):
                item = next(w3_iter, None)
                if item is None:
                    return
                i, ch, cw = item
                def store(qt, i=i, ch=ch, cw=cw):
                    for jb in range(cw // 128):
                        hb = ch // 128 + jb
                        nc.gpsimd.dma_start(
                            s3[hb, :, i * 128 : (i + 1) * 128],
                            qt[:, jb * 128 : (jb + 1) * 128],
                        )
                dq_push(w3s, i * 128, ch, cw, store)

        # ---------------- phase 2 ----------------
        def load_x(tb):
            xk = []
            for k in range(KT):
                xf = p.pxb.tile([128, TBP], BF16, tag="xb", name="xb")
                nc.sync.dma_start(
                    xf[:], xT[k * 128 : (k + 1) * 128, tb * TBP : (tb + 1) * TBP]
                )
                xk.append(xf)
            return xk

        def load_strip(s, h, tag):
            segs = []
            for k0 in range(0, KT * 128, SEG):
                seg = p.pl.tile([128, SEG], BF16, tag=tag, name=tag)
                nc.sync.dma_start(
                    seg[:],
                    s[h, :, k0 // 128 : (k0 + SEG) // 128, :].rearrange(
                        "p k i -> p (k i)"
                    ),
                )
                segs.append(seg)
            return segs

        def lhs_slice(segs, k):
            o = (k * 128) % SEG
            return segs[(k * 128) // SEG][:, o : o + 128]

        def phase2_pair(pair, h, xka, xkb):
            l1 = load_strip(s1, h, "l1")
            l2 = load_strip(s2, h, "l2")
            pg_a = p.pps.tile([128, TBP], F32, tag="ps", name="pg_a")
            pg_b = p.pps.tile([128, TBP], F32, tag="ps", name="pg_b")
            for k in range(KT):
                sl_ap = lhs_slice(l1, k)
                nc.tensor.matmul(pg_a[:], sl_ap, xka[k][:], start=(k == 0), stop=(k == KT - 1))
                nc.tensor.matmul(pg_b[:], sl_ap, xkb[k][:], start=(k == 0), stop=(k == KT - 1))
            # silu immediately so the gate PSUM banks free before the up k-loop
            sl_a = p.psl.tile([128, TBP], BF16, tag="sl", name="sl_a")
            nc.scalar.activation(sl_a[:], pg_a[:], AF.Silu)
            sl_b = p.psl.tile([128, TBP], BF16, tag="sl", name="sl_b")
            nc.scalar.activation(sl_b[:], pg_b[:], AF.Silu)
            pu_a = p.pps.tile([128, TBP], F32, tag="ps", name="pu_a")
            pu_b = p.pps.tile([128, TBP], F32, tag="ps", name="pu_b")
            for k in range(KT):
                sl_ap = lhs_slice(l2, k)
                nc.tensor.matmul(pu_a[:], sl_ap, xka[k][:], start=(k == 0), stop=(k == KT - 1))
                nc.tensor.matmul(pu_b[:], sl_ap, xkb[k][:], start=(k == 0), stop=(k == KT - 1))
            for slt, pu, tb in ((sl_a, pu_a, 2 * pair), (sl_b, pu_b, 2 * pair + 1)):
                ue = p.pue.tile([128, TBP], BF16, tag="ue", name="ue")
                nc.scalar.copy(ue[:], pu[:])
                ht = p.pht.tile([128, TBP], BF16, tag="ht", name="ht")
                nc.gpsimd.tensor_tensor(ht[:], slt[:], ue[:], OP.mult)
                nc.gpsimd.dma_start(hTd[h, :, tb * TBP : (tb + 1) * TBP], ht[:])

        # ---------------- phase 3 ----------------
        def phase3(tb3):
            strips = []
            for k in range(HT):
                hl = p.phl.tile([128, T3], BF16, tag="hl", name="hl")
                nc.sync.dma_start(hl[:], hTd[k, :, tb3 * T3 : (tb3 + 1) * T3])
                strips.append(hl)
            for dc in range(D // 512):
                r3s = []
                for k in range(HT):
                    r3 = p.pr3.tile([128, 512], BF16, tag="r3", name="r3")
                    nc.sync.dma_start(r3[:], s3[k, :, dc * 512 : (dc + 1) * 512])
                    r3s.append(r3)
                for th in range(2):
                    po = [
                        p.pps.tile([128, 512], F32, tag="ps", name=f"po{tt}")
                        for tt in range(4)
                    ]
                    for k in range(HT):
                        for i in range(4):
                            tt = th * 4 + i
                            nc.tensor.matmul(
                                po[i][:],
                                strips[k][:, tt * 128 : (tt + 1) * 128],
                                r3s[k][:],
                                start=(k == 0), stop=(k == HT - 1),
                            )
                    for i in range(4):
                        tt = th * 4 + i
                        ob = p.pob.tile([128, 512], F32, tag="ob", name="ob")
                        nc.scalar.copy(ob[:], po[i][:])
                        nc.gpsimd.dma_start(
                            out[
                                tb3 * T3 + tt * 128 : tb3 * T3 + (tt + 1) * 128,
                                dc * 512 : (dc + 1) * 512,
                            ],
                            ob[:],
                        )

        # ---------------- main flow ----------------
        # w3 dequant is front-loaded into pairs 1-2 so s3 completes before
        # pair 3; phase3(0,1) then interleaves ahead of pair 3's matmuls.
        # Lag-1 emission: each h's phase2 is emitted BEFORE the next h's
        # dequant batch, so the silu/ue PSUM evictions sit ahead of the
        # dequant ops in the ACT queue and PSUM banks recycle promptly.
        # All w3 dequant is packed into pair 1 so s3 completes early, and
        # phase3 blocks are interleaved between pairs 2/3 to keep the PE
        # busy while the dequant chain drains.
        w3_per_pair = {1: 4, 2: 2, 3: 0}
        prev = None  # (pair, h, xka, xkb)
        for pair in range(NPAIR):
            xka = load_x(2 * pair)
            xkb = load_x(2 * pair + 1)
            for h in range(HT):
                if pair == 0:
                    dq_w12(1, s1, h)
                    dq_w12(2, s2, h)
                    if h == HT - 1:
                        dq_drain()  # finish last tiles before pair 1 strips
                else:
                    emit_w3(w3_per_pair[pair])
                if prev is not None:
                    phase2_pair(*prev)
                prev = (pair, h, xka, xkb)
                if pair == 3 and h == 5:
                    phase3(2)
            if pair == 2:
                emit_w3(len(w3_work))  # drain any remainder
                dq_drain()
                phase2_pair(*prev)
                prev = None
                phase3(0)
                phase3(1)
        phase2_pair(*prev)
        phase3(3)

    nc.compile()
    return nc


_CACHED_NC = None
LAST_RESULTS = None


def _shard_inputs(x, w1, w2, w3):
    xT16 = np.ascontiguousarray(
        x.reshape(T_FULL, D).T.astype(ml_dtypes.bfloat16)
    )
    in_maps = []
    for c in range(N_CORES):
        s, w = SHARD_START[c], SHARD_W[c]
        w1c = np.zeros((HP, D), dtype=np.float32)
        w1c[:w] = w1[s : s + w]
        w2c = np.zeros((HP, D), dtype=np.float32)
        w2c[:w] = w2[s : s + w]
        w3c = np.zeros((D, HP), dtype=np.float32)
        w3c[:, :w] = w3[:, s : s + w]
        in_maps.append({"xT": xT16, "w1s": w1c, "w2s": w2c, "w3s": w3c})
    return in_maps


def kernel(x, w1, w2, w3):
    global _CACHED_NC, LAST_RESULTS
    assert x.shape == (2, 2048, D) and w1.shape == (H_FULL, D)
    if _CACHED_NC is None:
        _CACHED_NC = _build_program()
    in_maps = _shard_inputs(x, w1, w2, w3)
    res = run_bass_kernel_spmd(
        _CACHED_NC,
        in_maps,
        core_ids=list(range(N_CORES)),
        trace=os.environ.get("KERNEL_TRACE", "") == "1",
    )
    LAST_RESULTS = res
    acc = res.results[0]["out"].astype(np.float32).copy()
    for c in range(1, N_CORES):
        acc += res.results[c]["out"]
    return acc.reshape(2, 2048, D).astype(np.float32)


# revision 40
# speedup vs baseline: 1.0354x; 1.0354x over previous
"""Bass/Trainium2 kernel for NF4-dequant (QLoRA-style) SwiGLU MLP.

Computation (matches the bitsandbytes-NF4 reference):
    dq_i = nf4_quant_dequant(w_i)   (per-64-block absmax scaling)
    out  = dq3-proj( silu(x @ dq1^T) * (x @ dq2^T) )

Sharding: tensor-parallel over the ffn dim H=11008 across 8 cores.
H is split in 64-aligned shards [1408 x4, 1344 x4] (zero-padded to 1408
so every core runs the same program); each core emits a full [T, D]
partial of the down-projection and the host sums the 8 partials (the
TP all-reduce).  x is fed pre-transposed in bf16.

Dequant pipeline per [128 x cw] natural weight tile (software-pipelined
stage1/stage2 with lag-3):
  S1: 4-way-split wt DMA; absmax block-reduce, clamped reciprocal (DVE);
      one fused STT normalize vn = (w*32760)*(1/absmax) -> int16.
  S2: the 15-boundary NF4 bucketize as sum_j step_j * IDELT_j where all
      step terms are 16-bit mask tiles:
        - 10 one-op is_gt compares on DVE ({0,1} bf16)
        - 5  Sign activations on ACT (exact {-1,+1}; integer bounds with
          a +0.5 bias make ties impossible)
      and ALL folds run on the otherwise-idle PE as tiny stationary
      matmuls  psum += diag(IDELT_j or IDELT_j/2) @ mask_j  accumulated
      in f32 PSUM (no int16 add chain on DVE -- each DVE op costs ~2x
      its slice time due to the pipe DRAIN, so op count is what
      matters).  One fused STT per 512-half rescales
      (psum - OFFSET) * absmax/16384 -> bf16, XBAR DMA transposes
      (SBUF->SBUF on the SP hwdge queue) produce lhsT-layout tiles, and
      one batched DMA stores them to DRAM scratch.

Phase2 processes token-blocks in pairs (2 x 512): each s1/s2 strip load
feeds two blocks, each lhsT stationary feeds 2x512 columns, and silu
runs right after the gate k-loop so PSUM banks recycle early.  All w3
dequant is interleaved into pairs 1-2; phase3 blocks 0/1 run between
pairs, block 2 mid-pair-3, so most of the down-projection overlaps the
dequant chain.

Env knobs: KSPLIT="d,g,a" (# compares on DVE/GPSIMD/ACT; g must be 0 --
gpsimd turned out to be useless for both compares and int16 adds).
"""

import os
import sys

import numpy as np

if not os.path.isdir(os.path.join(os.path.dirname(os.path.abspath(__file__)), "concourse")):
    for _p in ("/opt/trn_rl_repo",):
        if os.path.isdir(_p) and _p not in sys.path:
            sys.path.insert(0, _p)

import ml_dtypes

import concourse.bass as bass
import concourse.mybir as mybir
import concourse.tile as tile
from concourse import bacc
from concourse.bass_utils import run_bass_kernel_spmd

F32 = mybir.dt.float32
F16 = mybir.dt.float16
BF16 = mybir.dt.bfloat16
I16 = mybir.dt.int16
OP = mybir.AluOpType
AF = mybir.ActivationFunctionType

NF4_CODE = np.array(
    [
        -1.0, -0.6961928009986877, -0.5250730514526367, -0.39491748809814453,
        -0.28444138169288635, -0.18477343022823334, -0.09105003625154495, 0.0,
        0.07958029955625534, 0.16093020141124725, 0.24611230194568634,
        0.33791524171829224, 0.44070982933044434, 0.5626170039176941,
        0.7229568362236023, 1.0,
    ],
    dtype=np.float64,
)
NF4_BOUNDS = (NF4_CODE[:-1] + NF4_CODE[1:]) * 0.5

CSCALE = 32760.0  # int16 compare-domain scale (saturation-safe)
IBOUND = [int(np.floor(b * CSCALE)) for b in NF4_BOUNDS]
# Even cumulative code table at scale 16384: CUM[j] ~ (code_j + 1) * 16384.
CUM = [2 * int(round((c + 1.0) * 8192.0)) for c in NF4_CODE]
IDELT = [CUM[j + 1] - CUM[j] for j in range(15)]  # all even
VSCALE = 16384.0

BLK = 64

D = 4096
T_FULL = 4096
H_FULL = 11008
N_CORES = 8
HP = 1408
SHARD_W = [1408, 1408, 1408, 1408, 1344, 1344, 1344, 1344]
SHARD_START = [0, 1408, 2816, 4224, 5632, 6976, 8320, 9664]

KT = D // 128  # 32
HT = HP // 128  # 11
TBP = 512           # phase2 token block (processed in pairs)
NPAIR = T_FULL // (2 * TBP)  # 4
T3 = 1024           # phase3 token block
SEG = 2048          # strip segment width (bf16)

DQ_CHUNK = 1024
W3_CHUNKS = [(0, 1024), (1024, 384)]

_split = os.environ.get("KSPLIT", "10,0,5")
N_DVE, N_GPS, N_ACT = [int(v) for v in _split.split(",")]
assert N_DVE + N_GPS + N_ACT == 15
DVE_J = list(range(0, N_DVE))
GPS_J = list(range(N_DVE, N_DVE + N_GPS))
ACT_J = list(range(N_DVE + N_GPS, 15))
S_HALF = sum(IDELT[j] // 2 for j in ACT_J)
OFFSET = 16384 - S_HALF  # dq = (acc - OFFSET) * absmax/16384

# Pool (gpsimd) integer ops require matching dtypes on all operands, so the
# f32->i16 normalize and i16xf32->bf16 rescale must stay on DVE.
GPS_NORM = os.environ.get("KERNEL_GPS_NORM", "0") == "1"
GPS_RESCALE = os.environ.get("KERNEL_GPS_RESCALE", "0") == "1"
# gpsimd tensor_reduce is partition-axis only; free-dim absmax must run on DVE
GPS_REDUCE = os.environ.get("KERNEL_GPS_REDUCE", "0") == "1"


class P:
    pass


def _build_program():
    nc = bacc.Bacc("TRN2", target_bir_lowering=False, debug=False, num_devices=N_CORES)

    xT = nc.dram_tensor("xT", [D, T_FULL], BF16, kind="ExternalInput").ap()
    w1s = nc.dram_tensor("w1s", [HP, D], F32, kind="ExternalInput").ap()
    w2s = nc.dram_tensor("w2s", [HP, D], F32, kind="ExternalInput").ap()
    w3s = nc.dram_tensor("w3s", [D, HP], F32, kind="ExternalInput").ap()
    out = nc.dram_tensor("out", [T_FULL, D], F32, kind="ExternalOutput").ap()

    from contextlib import ExitStack

    with tile.TileContext(nc) as tc, ExitStack() as ctx:
        p = P()
        dram = ctx.enter_context(tc.tile_pool(name="dram", bufs=1, space="DRAM"))
        s1 = dram.tile([HT, 128, KT, 128], BF16)
        s2 = dram.tile([HT, 128, KT, 128], BF16)
        s3 = dram.tile([HT, 128, D], BF16)
        hTd = dram.tile([HT, 128, T_FULL], BF16)

        const = ctx.enter_context(tc.tile_pool(name="const", bufs=1))
        bias_aps = []
        for j in range(15):
            b = const.tile([128, 1], F32, tag=f"bias{j}", name=f"bias{j}")
            nc.vector.memset(b[:], -(IBOUND[j] + 0.5))
            bias_aps.append(b)
        from concourse.masks import make_identity
        ident = const.tile([128, 128], F16, tag="ident", name="ident")
        make_identity(nc, ident[:])
        diag_aps = {}
        for j in range(15):
            val = IDELT[j] // 2 if j in ACT_J else IDELT[j]
            dgt = const.tile([128, 128], F16, tag=f"diag{j}", name=f"diag{j}")
            nc.vector.tensor_scalar_mul(dgt[:], ident[:], float(val))
            diag_aps[j] = dgt

        pool_spec = [
            ("pxb", 64, [128, TBP], BF16),
            ("pw", 5, [128, DQ_CHUNK], F32),
            ("pa", 6, [128, DQ_CHUNK // BLK], F32),
            ("pvn", 5, [128, DQ_CHUNK], I16),
            ("pmask", 9, [128, DQ_CHUNK], BF16),
            ("pdq", 5, [128, DQ_CHUNK], BF16),
            ("pqt", 5, [128, DQ_CHUNK], BF16),
            ("pl", 3, [128, SEG], BF16),  # bufs are per-tag (l1, l2)
            ("pht", 2, [128, TBP], BF16),
            ("psl", 3, [128, TBP], BF16),
            ("pue", 2, [128, TBP], BF16),
            ("phl", 11, [128, T3], BF16),
            ("pr3", 12, [128, 512], BF16),
            ("pob", 2, [128, 512], F32),
        ]
        for nm, bufs, shape, dt in pool_spec:
            setattr(p, nm, ctx.enter_context(tc.tile_pool(name=nm, bufs=bufs)))
        p.pps = ctx.enter_context(tc.tile_pool(name="pps", bufs=8, space="PSUM"))

        # ---------------- dequant one [128, cw] natural tile ----------------
        # Stage 1 (loads + absmax + normalize) for tile n+1 is emitted before
        # stage 2 (compares/folds/rescale/store) of tile n, so ACT/PE see vn
        # one tile ahead of the DVE chain (software pipelining).
        def dq_tile_s1(w_ap, row0, col0, cw):
            nblk = cw // BLK
            wt = p.pw.tile([128, cw], F32, tag="wt", name="wt")
            qw = max(256, cw // 4)
            for c0 in range(0, cw, qw):
                ce = min(c0 + qw, cw)
                nc.sync.dma_start(
                    wt[:, c0:ce],
                    w_ap[row0 : row0 + 128, col0 + c0 : col0 + ce],
                )
            w3v = wt[:].rearrange("p (b i) -> p b i", i=BLK)

            amax = p.pa.tile([128, nblk], F32, tag="amax", name="amax")
            nc.vector.tensor_reduce(
                amax[:], w3v, axis=mybir.AxisListType.X, op=OP.max,
                apply_absolute_value=True,
            )
            aclamp = p.pa.tile([128, nblk], F32, tag="aclamp", name="aclamp")
            nc.vector.tensor_scalar_max(aclamp[:], amax[:], 1e-30)
            recip = p.pa.tile([128, nblk], F32, tag="recip", name="recip")
            nc.vector.reciprocal(recip[:], aclamp[:])
            # av must stay f32: amax/16384 ~ 3e-7 underflows fp16 normals
            av = p.pa.tile([128, nblk], F32, tag="av", name="av")
            nc.vector.tensor_scalar_mul(av[:], amax[:], 1.0 / VSCALE)

            r_b = recip[:].unsqueeze(2).broadcast_to([128, nblk, BLK])

            vn = p.pvn.tile([128, cw], I16, tag="vn", name="vn")
            vn3 = vn[:].rearrange("p (b i) -> p b i", i=BLK)
            # vn = (w * CSCALE) * (1/absmax), fused in one STT pass
            nc.vector.scalar_tensor_tensor(
                vn3, w3v, CSCALE, r_b, OP.mult, OP.mult
            )
            return (vn, av, cw)

        def dq_tile_s2(s1ctx, store_fn):
            vn, av, cw = s1ctx
            nblk = cw // BLK
            av_b = av[:].unsqueeze(2).broadcast_to([128, nblk, BLK])

            halves = [(h0, min(512, cw - h0)) for h0 in range(0, cw, 512)]
            fold_ps = [
                p.pps.tile([128, hw], F32, tag="ps", name="fold")
                for h0, hw in halves
            ]
            nterm = 15

            # DVE terms: one-op is_gt -> {0,1} bf16 masks, folded on PE with
            # diag(IDELT_j); ACT terms: Sign -> {-1,+1}, diag(IDELT_j/2).
            # Every fold is a PE matmul accumulating into PSUM; the DVE int16
            # accumulator (and its drain-heavy add chain) is gone.
            ti = 0
            for j in DVE_J:
                m = p.pmask.tile([128, cw], BF16, tag="mask", name=f"d{j}")
                nc.vector.tensor_scalar(
                    m[:], vn[:], float(IBOUND[j]), None, OP.is_gt
                )
                for (h0, hw), ps in zip(halves, fold_ps):
                    nc.tensor.matmul(
                        ps[:], diag_aps[j][:], m[:, h0 : h0 + hw],
                        start=(ti == 0), stop=(ti == nterm - 1),
                    )
                ti += 1
            for j in ACT_J:
                sg = p.pmask.tile([128, cw], BF16, tag="mask", name=f"a{j}")
                nc.scalar.activation(sg[:], vn[:], AF.Sign, bias=bias_aps[j][:])
                for (h0, hw), ps in zip(halves, fold_ps):
                    nc.tensor.matmul(
                        ps[:], diag_aps[j][:], sg[:, h0 : h0 + hw],
                        start=(ti == 0), stop=(ti == nterm - 1),
                    )
                ti += 1

            dq = p.pdq.tile([128, cw], BF16, tag="dq", name="dq")
            for (h0, hw), ps in zip(halves, fold_ps):
                nb0, nbw = h0 // BLK, hw // BLK
                dqv = dq[:, h0 : h0 + hw].rearrange("p (b i) -> p b i", i=BLK)
                psv = ps[:].rearrange("p (b i) -> p b i", i=BLK)
                nc.vector.scalar_tensor_tensor(
                    dqv, psv, float(-OFFSET), av_b[:, nb0 : nb0 + nbw, :],
                    OP.add, OP.mult,
                )
            qt = p.pqt.tile([128, cw], BF16, tag="qt", name="qt")
            for jb in range(cw // 128):
                sl = slice(jb * 128, (jb + 1) * 128)
                nc.sync.dma_start_transpose(qt[:, sl], dq[:, sl])
            store_fn(qt)

        dq_pending = []

        def dq_push(w_ap, row0, col0, cw, store_fn):
            s1ctx = dq_tile_s1(w_ap, row0, col0, cw)
            dq_pending.append((s1ctx, store_fn))
            if len(dq_pending) > 3:
                dq_tile_s2(*dq_pending.pop(0))

        def dq_drain():
            while dq_pending:
                dq_tile_s2(*dq_pending.pop(0))

        def dq_w12(which, s, h):
            w_ap = w1s if which == 1 else w2s
            for ch in range(0, D, DQ_CHUNK):
                def store(qt, ch=ch, h=h, s=s):
                    kt0 = ch // 128
                    nkt = DQ_CHUNK // 128
                    dst = s[h, :, kt0 : kt0 + nkt, :].rearrange("p k i -> p (k i)")
                    nc.gpsimd.dma_start(dst, qt[:])
                dq_push(w_ap, h * 128, ch, DQ_CHUNK, store)

        w3_work = [(i, ch, cw) for i in range(KT) for (ch, cw) in W3_CHUNKS]
        w3_iter = iter(w3_work)

        def emit_w3(n):
            for _ in range(n):
                item = next(w3_iter, None)
                if item is None:
                    return
                i, ch, cw = item
                def store(qt, i=i, ch=ch, cw=cw):
                    for jb in range(cw // 128):
                        hb = ch // 128 + jb
                        nc.gpsimd.dma_start(
                            s3[hb, :, i * 128 : (i + 1) * 128],
                            qt[:, jb * 128 : (jb + 1) * 128],
                        )
                dq_push(w3s, i * 128, ch, cw, store)

        # ---------------- phase 2 ----------------
        def load_x(tb):
            xk = []
            for k in range(KT):
                xf = p.pxb.tile([128, TBP], BF16, tag="xb", name="xb")
                nc.sync.dma_start(
                    xf[:], xT[k * 128 : (k + 1) * 128, tb * TBP : (tb + 1) * TBP]
                )
                xk.append(xf)
            return xk

        def load_strip(s, h, tag):
            segs = []
            for k0 in range(0, KT * 128, SEG):
                seg = p.pl.tile([128, SEG], BF16, tag=tag, name=tag)
                nc.sync.dma_start(
                    seg[:],
                    s[h, :, k0 // 128 : (k0 + SEG) // 128, :].rearrange(
                        "p k i -> p (k i)"
                    ),
                )
                segs.append(seg)
            return segs

        def lhs_slice(segs, k):
            o = (k * 128) % SEG
            return segs[(k * 128) // SEG][:, o : o + 128]

        def phase2_pair(pair, h, xka, xkb):
            l1 = load_strip(s1, h, "l1")
            l2 = load_strip(s2, h, "l2")
            pg_a = p.pps.tile([128, TBP], F32, tag="ps", name="pg_a")
            pg_b = p.pps.tile([128, TBP], F32, tag="ps", name="pg_b")
            for k in range(KT):
                sl_ap = lhs_slice(l1, k)
                nc.tensor.matmul(pg_a[:], sl_ap, xka[k][:], start=(k == 0), stop=(k == KT - 1))
                nc.tensor.matmul(pg_b[:], sl_ap, xkb[k][:], start=(k == 0), stop=(k == KT - 1))
            # silu immediately so the gate PSUM banks free before the up k-loop
            sl_a = p.psl.tile([128, TBP], BF16, tag="sl", name="sl_a")
            nc.scalar.activation(sl_a[:], pg_a[:], AF.Silu)
            sl_b = p.psl.tile([128, TBP], BF16, tag="sl", name="sl_b")
            nc.scalar.activation(sl_b[:], pg_b[:], AF.Silu)
            pu_a = p.pps.tile([128, TBP], F32, tag="ps", name="pu_a")
            pu_b = p.pps.tile([128, TBP], F32, tag="ps", name="pu_b")
            for k in range(KT):
                sl_ap = lhs_slice(l2, k)
                nc.tensor.matmul(pu_a[:], sl_ap, xka[k][:], start=(k == 0), stop=(k == KT - 1))
                nc.tensor.matmul(pu_b[:], sl_ap, xkb[k][:], start=(k == 0), stop=(k == KT - 1))
            for slt, pu, tb in ((sl_a, pu_a, 2 * pair), (sl_b, pu_b, 2 * pair + 1)):
                ue = p.pue.tile([128, TBP], BF16, tag="ue", name="ue")
                nc.scalar.copy(ue[:], pu[:])
                ht = p.pht.tile([128, TBP], BF16, tag="ht", name="ht")
                nc.gpsimd.tensor_tensor(ht[:], slt[:], ue[:], OP.mult)
                nc.gpsimd.dma_start(hTd[h, :, tb * TBP : (tb + 1) * TBP], ht[:])

        # ---------------- phase 3 ----------------
        def phase3(tb3):
            strips = []
            for k in range(HT):
                hl = p.phl.tile([128, T3], BF16, tag="hl", name="hl")
                nc.sync.dma_start(hl[:], hTd[k, :, tb3 * T3 : (tb3 + 1) * T3])
                strips.append(hl)
            for dc in range(D // 512):
                r3s = []
                for k in range(HT):
                    r3 = p.pr3.tile([128, 512], BF16, tag="r3", name="r3")
                    nc.sync.dma_start(r3[:], s3[k, :, dc * 512 : (dc + 1) * 512])
                    r3s.append(r3)
                for th in range(2):
                    po = [
                        p.pps.tile([128, 512], F32, tag="ps", name=f"po{tt}")
                        for tt in range(4)
                    ]
                    for k in range(HT):
                        for i in range(4):
                            tt = th * 4 + i
                            nc.tensor.matmul(
                                po[i][:],
                                strips[k][:, tt * 128 : (tt + 1) * 128],
                                r3s[k][:],
                                start=(k == 0), stop=(k == HT - 1),
                            )
                    for i in range(4):
                        tt = th * 4 + i
                        ob = p.pob.tile([128, 512], F32, tag="ob", name="ob")
                        nc.scalar.copy(ob[:], po[i][:])
                        nc.gpsimd.dma_start(
                            out[
                                tb3 * T3 + tt * 128 : tb3 * T3 + (tt + 1) * 128,
                                dc * 512 : (dc + 1) * 512,
                            ],
                            ob[:],
                        )

        # ---------------- main flow ----------------
        # w3 dequant is front-loaded into pairs 1-2 so s3 completes before
        # pair 3; phase3(0,1) then interleaves ahead of pair 3's matmuls.
        # Lag-1 emission: each h's phase2 is emitted BEFORE the next h's
        # dequant batch, so the silu/ue PSUM evictions sit ahead of the
        # dequant ops in the ACT queue and PSUM banks recycle promptly.
        # All w3 dequant is packed into pair 1 so s3 completes early, and
        # phase3 blocks are interleaved between pairs 2/3 to keep the PE
        # busy while the dequant chain drains.
        w3_per_pair = {1: 4, 2: 2, 3: 0}
        prev = None  # (pair, h, xka, xkb)
        for pair in range(NPAIR):
            xka = load_x(2 * pair)
            xkb = load_x(2 * pair + 1)
            for h in range(HT):
                if pair == 0:
                    dq_w12(1, s1, h)
                    dq_w12(2, s2, h)
                    if h == HT - 1:
                        dq_drain()  # finish last tiles before pair 1 strips
                else:
                    emit_w3(w3_per_pair[pair])
                if prev is not None:
                    phase2_pair(*prev)
                prev = (pair, h, xka, xkb)
                if pair == 3 and h == 5:
                    phase3(2)
            if pair == 2:
                emit_w3(len(w3_work))  # drain any remainder
                dq_drain()
                phase2_pair(*prev)
                prev = None
                phase3(0)
                phase3(1)
        phase2_pair(*prev)
        phase3(3)

    nc.compile()
    return nc


_CACHED_NC = None
LAST_RESULTS = None


def _shard_inputs(x, w1, w2, w3):
    xT16 = np.ascontiguousarray(
        x.reshape(T_FULL, D).T.astype(ml_dtypes.bfloat16)
    )
    in_maps = []
    for c in range(N_CORES):
        s, w = SHARD_START[c], SHARD_W[c]
        w1c = np.zeros((HP, D), dtype=np.float32)
        w1c[:w] = w1[s : s + w]
        w2c = np.zeros((HP, D), dtype=np.float32)
        w2c[:w] = w2[s : s + w]
        w3c = np.zeros((D, HP), dtype=np.float32)
        w3c[:, :w] = w3[:, s : s + w]
        in_maps.append({"xT": xT16, "w1s": w1c, "w2s": w2c, "w3s": w3c})
    return in_maps


def kernel(x, w1, w2, w3):
    global _CACHED_NC, LAST_RESULTS
    assert x.shape == (2, 2048, D) and w1.shape == (H_FULL, D)
    if _CACHED_NC is None:
        _CACHED_NC = _build_program()
    in_maps = _shard_inputs(x, w1, w2, w3)
    res = run_bass_kernel_spmd(
        _CACHED_NC,
        in_maps,
        core_ids=list(range(N_CORES)),
        trace=os.environ.get("KERNEL_TRACE", "") == "1",
    )
    LAST_RESULTS = res
    acc = res.results[0]["out"].astype(np.float32).copy()
    for c in range(1, N_CORES):
        acc += res.results[c]["out"]
    return acc.reshape(2, 2048, D).astype(np.float32)
